# revision 52
# baseline (speedup 1.0000x reference)
"""BarrierNet forward on 8 Trainium2 NeuronCores (pure batch data-parallel).

Math actually needed (x32 / x0 branches of the reference are dead code):
    h   = relu(x @ W1 + b1)                       [B, 2048]
    a   = relu(h @ W21 + b21)                     [B, 1024]
    t   = a @ W31                                 [B, 2]    (bias folded below)
    out = clip(-t + bias2, lo2, hi2)              [B, 2]
with host-folded per-channel constants
    bias2 = -(b31 + 2*om/os),  lo2 = (lo-om)/os,  hi2 = (hi-om)/os
    lo = [-(1+s3), -(1+s1)],   hi = [1+s2, 1+s0]

Device dataflow keeps features on the partition dim (x^T -> h^T -> a^T ->
x31^T) so every weight matrix is used directly as the stationary lhsT and
only the tiny x / out tensors ever need a transpose (done on host).

Performance notes (measured on TRN2, bf16, trace-on HW exec ~283us vs
438us fp32r baseline; PE streams at its 216ns/512-col floor >98% of the
time):
  - bf16 matmuls: 216ns per 128x128x512 vs ~306ns fp32r on real HW.
  - mm1/mm3 stationaries zero-padded to 128 rows/cols: a PE tile-config
    switch (e.g. (32,128) <-> (128,128)) costs ~250ns per transition.
  - mm1 tiles are interleaved into mm2's m-tile stream (2 per m-tile, plus
    a k-paced self-fill for chunk 0) because the two PSUM-capable act
    engines (~700ns per [128,512] evac; GPSIMD cannot read PSUM) can't
    drain a 16-tile mm1 burst.
  - ~70 dummy matmuls on a zeroed tile warm the PE p-state during the
    ~10us before the first input DMA lands (engines boot at ~+6us; PE
    runs ~2x slow for its first ~3us of activity).
  - Output staged [N_CL, B_SH] in SBUF, stored per chunk as 2 contiguous
    descriptors, triggered by the scalar engine (a sync-engine trigger for
    the last chunk sits behind ~7us of semaphore-teardown chatter).
"""

import os

import numpy as np

B, N_IN, H1, H2, N_CL = 32768, 8, 2048, 1024, 2
N_CORES = 8
B_SH = B // N_CORES  # 4096 rows per core
NB = 512             # batch-chunk width (matmul free dim / PSUM bank)
N_CHUNKS = B_SH // NB
MT1 = H1 // 128      # 16 output tiles of mm1
KT2, MT2 = H1 // 128, H2 // 128  # 16 k-tiles, 8 m-tiles of mm2
KT3 = H2 // 128      # 8 k-tiles of mm3

MM_MODE = os.environ.get("BARRIER_MM_MODE", "bf16")  # fp32r | bf16 | fp32
TRACE = bool(int(os.environ.get("BARRIER_TRACE", "0")))

_CACHE = {}
last_results = None  # BassKernelResults of the most recent run (for test.py)


def _build(mode):
    from contextlib import ExitStack

    import concourse.bass as bass
    import concourse.mybir as mybir
    import concourse.tile as tile
    from concourse import bacc

    f32 = mybir.dt.float32
    if mode == "bf16":
        io_dt = mybir.dt.bfloat16
    elif mode == "fp32r":
        io_dt = mybir.dt.float32r
    else:
        io_dt = f32

    def mm(ap):
        return ap

    nc = bacc.Bacc("TRN2", debug=False, num_devices=N_CORES)

    # xT / w1 are zero-padded K=8 -> K=128 on host: keeping every matmul at
    # tile_size (128,128) avoids the PE reconfiguration penalty (~250ns per
    # mm1<->mm2 transition measured) at the cost of streaming zeros on the
    # contraction dim (free: matmul cost is independent of K).
    xT_d = nc.dram_tensor("xT", [128, B_SH], io_dt, kind="ExternalInput").ap()
    w1_d = nc.dram_tensor("w1", [128, H1], io_dt, kind="ExternalInput").ap()
    w21_d = nc.dram_tensor("w21", [H1, H2], io_dt, kind="ExternalInput").ap()
    w31_d = nc.dram_tensor("w31", [H2, N_CL], io_dt, kind="ExternalInput").ap()
    b1_d = nc.dram_tensor("b1", [H1], f32, kind="ExternalInput").ap()
    b21_d = nc.dram_tensor("b21", [H2], f32, kind="ExternalInput").ap()
    post_d = nc.dram_tensor("post", [N_CL, 3], f32, kind="ExternalInput").ap()
    # Output kept transposed [N_CL, B_SH] so each chunk's store is 2 fully
    # contiguous rows (2 DMA descriptors) instead of 512 8-byte strided
    # descriptors; host transposes the 32KB result.
    out_d = nc.dram_tensor("out", [N_CL, B_SH], f32, kind="ExternalOutput").ap()

    Relu = mybir.ActivationFunctionType.Relu
    Ident = mybir.ActivationFunctionType.Identity
    add_op = mybir.AluOpType.add
    max_op = mybir.AluOpType.max
    min_op = mybir.AluOpType.min

    with tile.TileContext(nc) as tc, ExitStack() as ctx:
        const = ctx.enter_context(tc.tile_pool(name="const", bufs=1))
        wpool = ctx.enter_context(tc.tile_pool(name="w21", bufs=1))
        hpool = ctx.enter_context(tc.tile_pool(name="hT", bufs=3))
        apool = ctx.enter_context(tc.tile_pool(name="aT", bufs=1))
        # ps_h gets 4 banks (mm1 pairs are issued back-to-back and the
        # chunk-0 self-fill runs one tile per mm2 k-step — the extra slot
        # absorbs act latency); ps_a only needs 3: its rotation window is
        # ~3 m-tiles (~10us) vs ~750ns evac latency, so it never blocks.
        ps_h = ctx.enter_context(tc.tile_pool(name="ps_h", bufs=4, space="PSUM"))
        ps_a = ctx.enter_context(tc.tile_pool(name="ps_a", bufs=3, space="PSUM"))
        ps_o = ctx.enter_context(tc.tile_pool(name="ps_o", bufs=1, space="PSUM"))

        # Stationary weights / constants.
        # DMA issue order is the critical path: SP issues serially (~0.6us
        # each) and nothing lands before ~12us of NEFF startup. Put exactly
        # what the first matmuls + evacs need first: xT/w1, then b1;
        # remaining constants; then the 8MB W21.
        # (tile_position row-group packing was measured to give zero
        # concurrency — the PE serializes instructions — so mm1 just uses
        # rows 0:8 of the array.)
        # PE p-state warmup first: dummy matmuls on a zeroed tile while the
        # input DMAs land (DMA queues take ~5.4us to issue anything, but the
        # compute engines start within ~0.2us; the PE ramps to full clock
        # only after ~3us of activity, so idling the head costs ~2x on the
        # first chunk).
        wz = const.tile([128, 128], io_dt)
        nc.vector.memset(wz, 0.0)
        pw = ps_h.tile([128, NB], f32, tag="ph")
        for _ in range(70):
            nc.tensor.matmul(
                pw[:, 0:64], mm(wz), mm(wz[:, 0:64]), start=True, stop=True
            )

        w1_sb = const.tile([128, H1], io_dt)
        xT_sb = const.tile([128, B_SH], io_dt)
        nc.sync.dma_start(out=xT_sb[:, 0:NB], in_=xT_d[:, 0:NB])
        nc.sync.dma_start(out=w1_sb[:, 0:384], in_=w1_d[:, 0:384])
        b1_sb = const.tile([128, MT1], f32)
        nc.sync.dma_start(out=b1_sb, in_=b1_d.rearrange("(k p) -> p k", p=128))
        nc.sync.dma_start(out=w1_sb[:, 384:], in_=w1_d[:, 384:])
        w21_t = []
        for k in range(KT2):
            t = wpool.tile([128, H2], io_dt, tag=f"w21_{k}")
            nc.sync.dma_start(out=t, in_=w21_d[k * 128 : (k + 1) * 128, :])
            w21_t.append(t)
        nc.sync.dma_start(out=xT_sb[:, NB : 2 * NB], in_=xT_d[:, NB : 2 * NB])
        nc.sync.dma_start(out=xT_sb[:, 2 * NB :], in_=xT_d[:, 2 * NB :])
        b21_sb = const.tile([128, MT2], f32)
        nc.sync.dma_start(out=b21_sb, in_=b21_d.rearrange("(k p) -> p k", p=128))
        post_sb = const.tile([N_CL, 3], f32)
        nc.sync.dma_start(out=post_sb, in_=post_d)
        # Output staged in SBUF, stored per chunk (2 contiguous descriptors).
        v_all = const.tile([N_CL, B_SH], f32)
        # W31 zero-padded [128,2] -> [128,128] stationaries so mm3 keeps the
        # same (128,128) PE tile config as mm1/mm2 (a config switch costs
        # ~90-130ns on the next matmul). Pad rows of the psum output are
        # garbage-free (zero weights) and simply never read.
        w31_sb = const.tile([128, KT3 * 128], io_dt)
        nc.vector.memset(w31_sb, 0.0)
        for k in range(KT3):
            nc.sync.dma_start(
                out=w31_sb[:, k * 128 : k * 128 + N_CL],
                in_=w31_d[k * 128 : (k + 1) * 128, :],
            )

        def mm1_tile(c, m):
            # One tile of hT = relu(W1^T @ xT + b1), K=8.
            bs = slice(c * NB, (c + 1) * NB)
            ph = ps_h.tile([128, NB], f32)
            nc.tensor.matmul(
                ph,
                mm(w1_sb[:, m * 128 : (m + 1) * 128]),
                mm(xT_sb[:, bs]),
                start=True,
                stop=True,
            )
            ht = hpool.tile([128, NB], io_dt, tag=f"h{m}")
            if m % 2 == 0:
                nc.scalar.activation(ht, ph, Relu, bias=b1_sb[:, m : m + 1])
            else:
                nc.vector.tensor_scalar(
                    out=ht, in0=ph, scalar1=b1_sb[:, m : m + 1],
                    scalar2=0.0, op0=add_op, op1=max_op,
                )
            return ht

        def mm23(c, hT, lead, self_fill=False):
            # lead: chunk index whose mm1 tiles get interleaved between this
            # chunk's mm2 m-tiles (2 per m-tile), so the two act engines see
            # a ~3.5us window per mm1 pair instead of a 16-tile burst that
            # overruns PSUM and stalls the PE.
            # self_fill (chunk 0 only): hT[3:] not produced yet; issue each
            # missing mm1 tile between this chunk's own m=0 k-steps, 3 steps
            # ahead of its consumption. (A 5-step lookahead with a 5-tile
            # prologue measured ~2.5us WORSE — the burst overruns ps_h.)
            bs = slice(c * NB, (c + 1) * NB)
            lead_hT = [None] * MT1 if lead is not None else None
            po = ps_o.tile([128, NB], f32)
            aT = []
            for m in range(MT2):
                pa = ps_a.tile([128, NB], f32)
                for k in range(KT2):
                    nc.tensor.matmul(
                        pa,
                        mm(w21_t[k][:, m * 128 : (m + 1) * 128]),
                        mm(hT[k]),
                        start=(k == 0),
                        stop=(k == KT2 - 1),
                    )
                    if self_fill and m == 0 and k + 3 < MT1 and hT[k + 3] is None:
                        hT[k + 3] = mm1_tile(c, k + 3)
                at = apool.tile([128, NB], io_dt, tag=f"a{m}")
                if m % 2 == 0:
                    nc.scalar.activation(at, pa, Relu, bias=b21_sb[:, m : m + 1])
                else:
                    nc.vector.tensor_scalar(
                        out=at, in0=pa, scalar1=b21_sb[:, m : m + 1],
                        scalar2=0.0, op0=add_op, op1=max_op,
                    )
                aT.append(at)
                if lead is not None:
                    for mm1_m in (2 * m, 2 * m + 1):
                        if lead_hT[mm1_m] is None:
                            lead_hT[mm1_m] = mm1_tile(lead, mm1_m)
                # mm3 k-step for the m-tile evacuated one iteration ago (its
                # act has had a full m-tile window to finish) — interleaving
                # mm3 into the m-loop pulls the chunk's postprocess + store
                # ~1.7us earlier instead of trailing all eight m-tiles.
                if m >= 1:
                    nc.tensor.matmul(
                        po,
                        mm(w31_sb[:, (m - 1) * 128 : m * 128]),
                        mm(aT[m - 1]),
                        start=(m == 1),
                        stop=False,
                    )
            # final mm3 k-step (last aT) + QP postprocess:
            # out = clip(-t + bias2, lo2, hi2)
            nc.tensor.matmul(
                po,
                mm(w31_sb[:, (KT3 - 1) * 128 : KT3 * 128]),
                mm(aT[KT3 - 1]),
                start=False,
                stop=True,
            )
            v = v_all[:, bs]
            nc.scalar.activation(v, po[0:N_CL, :], Ident, bias=post_sb[:, 0:1], scale=-1.0)
            nc.vector.tensor_scalar(
                out=v, in0=v, scalar1=post_sb[:, 1:2], scalar2=post_sb[:, 2:3],
                op0=max_op, op1=min_op,
            )
            # Per-chunk store (2 contiguous descriptors), triggered by the
            # scalar (Activation) engine whose queue is idle right after this
            # chunk's IDENT — a sync-engine trigger sits behind ~7us of
            # end-of-kernel semaphore teardown for the final chunk, and the
            # gpsimd SWDGE path measured ~3us slower. (Only
            # SP/Activation/gpsimd can initiate DMAs.)
            nc.scalar.dma_start(out=out_d[:, bs], in_=v)
            return lead_hT

        # Software pipeline: chunk 0 starts with only 3 mm1 tiles (the rest
        # self-fill inside its m=0 k-loop); chunk c+1's mm1 tiles are
        # produced inside mm23(c).
        h0 = [mm1_tile(0, m) for m in range(3)] + [None] * (MT1 - 3)
        hts = {0: h0}
        for c in range(N_CHUNKS):
            lead = c + 1 if c + 1 < N_CHUNKS else None
            lead_hT = mm23(c, hts.pop(c), lead, self_fill=(c == 0))
            if lead is not None:
                hts[lead] = lead_hT

    nc.compile()
    return nc


def _get_nc():
    if MM_MODE not in _CACHE:
        _CACHE[MM_MODE] = _build(MM_MODE)
    return _CACHE[MM_MODE]


def kernel(**inputs):
    global last_results
    from concourse.bass_utils import run_bass_kernel_spmd

    f32 = np.float32
    x = np.asarray(inputs["x"], f32)
    W1 = np.asarray(inputs["W1"], f32)
    b1 = np.ascontiguousarray(np.asarray(inputs["b1"], f32))
    W21 = np.asarray(inputs["W21"], f32)
    b21 = np.ascontiguousarray(np.asarray(inputs["b21"], f32))
    W31 = np.asarray(inputs["W31"], f32)
    b31 = np.asarray(inputs["b31"], f32)
    om = np.asarray(inputs["output_mean"], f32)
    os_ = np.asarray(inputs["output_std"], f32)
    s0 = np.asarray(inputs["s0"], f32)[0]
    s1 = np.asarray(inputs["s1"], f32)[0]
    s2 = np.asarray(inputs["s2"], f32)[0]
    s3 = np.asarray(inputs["s3"], f32)[0]

    lo = np.array([-(1.0 + s3), -(1.0 + s1)], f32)
    hi = np.array([1.0 + s2, 1.0 + s0], f32)
    bias2 = -(b31 + 2.0 * om / os_)
    post = np.ascontiguousarray(
        np.stack([bias2, (lo - om) / os_, (hi - om) / os_], axis=1).astype(f32)
    )

    if MM_MODE == "bf16":
        import ml_dtypes

        conv = lambda a: np.ascontiguousarray(a.astype(ml_dtypes.bfloat16))
    else:
        conv = lambda a: np.ascontiguousarray(a)
    w21c, w31c = conv(W21), conv(W31)
    w1c = conv(W1)
    w1p = np.zeros((128, H1), w1c.dtype)
    w1p[:N_IN] = w1c

    in_maps = []
    for c in range(N_CORES):
        xT = conv(x[c * B_SH : (c + 1) * B_SH].T)
        xTp = np.zeros((128, B_SH), xT.dtype)
        xTp[:N_IN] = xT
        in_maps.append(
            {"xT": xTp, "w1": w1p, "w21": w21c, "w31": w31c,
             "b1": b1, "b21": b21, "post": post}
        )

    nc = _get_nc()
    last_results = run_bass_kernel_spmd(
        nc, in_maps, list(range(N_CORES)), trace=TRACE
    )
    return np.concatenate(
        [last_results.results[c]["out"].T for c in range(N_CORES)], axis=0
    ).astype(f32)

